# revision 23
# baseline (speedup 1.0000x reference)
"""DeepseekV3 MLA attention on 8 Trainium2 NeuronCores (Bass/Tile).

Sharding: core = (batch b, head-group g); b = core//4, g = core%4.
Each core handles 4 of 16 heads for one batch.

 - q_a / kv_a low-rank projections are sequence-sharded within each
   4-core batch group (each core computes a 512-column chunk of
   q_a^T / kv_a^T).  The kv strip is AllGathered first; the q strip is
   gathered in TWO pieces, each issued as soon as its rows are evicted,
   so all three collectives complete while the Tensor engine is still
   busy with the remaining strips and the kv up-projection — no
   exposed collective time.
 - rms scale for the kv path depends only on the kv gather, so the kv
   up-projection overlaps the q gathers.
 - q_b / kv_b up-projections + causal attention computed per-core for
   its 4 heads, everything in transposed (feature-on-partition) layout.
 - o-proj partial output per core ([S, H] fp16, contraction over the
   512 local v-dims); host sums the 4 partials per batch.

All matmul operands are fp16, accumulation fp32 in PSUM.  Softmax uses
exp(scale*s - 2) with no row-max subtraction (cancels in the ratio).
The softmax denominator is accumulated on the Vector engine (fp16 adds
of probability tiles) with one ones-vector matmul per (head, Q-chunk)
for the final partition reduction, and the attention inner loop is
software-pipelined (scores of block j+1 issue before the AV matmul of
block j) so the Tensor engine does not stall on the Exp activation.
o-proj is interleaved per Q-chunk right after its 4 heads finish.
"""
import sys

sys.path.insert(0, "/opt/trn_rl_repo")

import numpy as np

import concourse.bass as bass
import concourse.tile as tile
from concourse import bacc, mybir
from concourse.bass_utils import run_bass_kernel_spmd

# ---- problem constants (hardcoded per contract) ----
B, S, H = 2, 2048, 2048
NH, D_NOPE, D_ROPE, D_V = 16, 128, 64, 128
D_QK = D_NOPE + D_ROPE
QLR, KVL = 1536, 512
SCALING = float(D_QK) ** -0.5
EXP_SHIFT = -2.0   # global exponent shift; cancels in softmax ratio
EPS = 1e-6
ROPE_THETA = 10000.0

NHL = 4            # heads per core
N_CORES = 8
SC = S // 4        # seq chunk per core for the a-projections (512)
P = 128
KV_ROWS = KVL + D_ROPE + 1   # 577: c_kv rows, k_rope rows, kv sumsq row
Q1_ROWS = 768                # first gathered q piece (f-tiles 0..5)
Q2_ROWS = QLR - Q1_ROWS + 1  # 769: f-tiles 6..11 plus the q sumsq row
# both pieces stay under the 1 MB AllGather input limit for the fast
# Mesh algorithm (the runtime falls back to Ring above it)

f32 = mybir.dt.float32
f32r = mybir.dt.float32r
f16 = mybir.dt.float16
AF = mybir.ActivationFunctionType


def _build_program():
    nc = bacc.Bacc("TRN2", target_bir_lowering=False, debug=False,
                   num_devices=N_CORES)

    def din(name, shape, dt=f16):
        return nc.dram_tensor(name, list(shape), dt, kind="ExternalInput").ap()

    # per-core inputs (host-prepped layouts, fp16)
    hT = din("hT", [H, SC])
    qawT_s = din("qawT_s", [16, 3, P, 512])      # q_a_w.T strips [k, mg, 128, 512]
    kvawT_s = din("kvawT_s", [16, P, 640])       # kv_a_w.T strips (cols padded 576->640)
    qbw_np = din("qbw_np", [QLR, NHL * D_NOPE])  # [1536, 512] own heads
    qbw_rp = din("qbw_rp", [QLR, NHL * D_ROPE])  # [1536, 256] own heads
    kvbw_k = din("kvbw_k", [KVL, NHL * D_NOPE])  # [512, 512] own heads
    kvbw_v = din("kvbw_v", [KVL, NHL * D_V])     # [512, 512] own heads
    owT = din("owT", [NHL * D_V, H])             # [512, 2048] own heads
    cosP = din("cosP", [P, S], f32)              # vstack(cos.T, cos.T)
    sinP = din("sinP", [P, S], f32)
    r128t = din("r128t", [P, P])                 # rotate-half matrix (transposed, block-diag)
    triu = din("triu", [P, P])                   # causal mask for diagonal blocks
    ones_col = din("ones_col", [P, 1])           # lhsT for denominator matmul
    ones_row = din("ones_row", [1, P], f32)      # lhsT for partition-broadcast matmul (f32r)

    o_part = nc.dram_tensor("o_part", [S, H], f16, kind="ExternalOutput").ap()

    # scratch
    warm_src = nc.dram_tensor("warm_src", [P, P], f16).ap()
    warm_dst = nc.dram_tensor("warm_dst", [4, P, P], f16).ap()
    a_kv_loc = nc.dram_tensor("a_kv_loc", [KV_ROWS, SC], f16).ap()
    a_kv = nc.dram_tensor("a_kv", [4, KV_ROWS, SC], f16).ap()
    a_q1_loc = nc.dram_tensor("a_q1_loc", [Q1_ROWS, SC], f16).ap()
    a_q1 = nc.dram_tensor("a_q1", [4, Q1_ROWS, SC], f16).ap()
    a_q2_loc = nc.dram_tensor("a_q2_loc", [Q2_ROWS, SC], f16).ap()
    a_q2 = nc.dram_tensor("a_q2", [4, Q2_ROWS, SC], f16).ap()
    r_dram = nc.dram_tensor("r_dram", [2, S], f32).ap()

    with tile.TileContext(nc) as tc:
        _emit(tc, nc, dict(
            hT=hT, qawT_s=qawT_s, kvawT_s=kvawT_s, qbw_np=qbw_np,
            qbw_rp=qbw_rp, kvbw_k=kvbw_k, kvbw_v=kvbw_v, owT=owT,
            cosP=cosP, sinP=sinP, r128t=r128t, triu=triu,
            ones_col=ones_col, ones_row=ones_row,
            o_part=o_part, a_kv_loc=a_kv_loc, a_kv=a_kv,
            a_q1_loc=a_q1_loc, a_q1=a_q1, a_q2_loc=a_q2_loc, a_q2=a_q2,
            r_dram=r_dram, warm_src=warm_src, warm_dst=warm_dst))
    nc.compile()
    return nc


def _emit(tc, nc, t):
    from contextlib import ExitStack

    dma = nc.sync.dma_start
    MM = nc.tensor.matmul
    GROUPS = [[0, 1, 2, 3], [4, 5, 6, 7]]

    def gather(src, dst):
        nc.gpsimd.collective_compute(
            "AllGather", mybir.AluOpType.bypass, replica_groups=GROUPS,
            ins=[src.opt()], outs=[dst.opt()])

    with ExitStack() as stack:
        ec = stack.enter_context

        # tiny dependency-free collective issued first: absorbs the one-time
        # cc-channel rendezvous barrier (~35us) before the real gathers need
        # the stream (gathered values are never read)
        gather(t["warm_src"], t["warm_dst"])

        consts = ec(tc.tile_pool(name="consts", bufs=1))
        ones_col_sb = consts.tile([P, 1], f16)
        dma(out=ones_col_sb, in_=t["ones_col"])
        ones_row_sb = consts.tile([1, P], f32r)
        dma(out=ones_row_sb, in_=t["ones_row"].bitcast(f32r))
        triu_sb = consts.tile([P, P], f16)
        dma(out=triu_sb, in_=t["triu"])
        r128_sb = consts.tile([P, P], f16)
        dma(out=r128_sb, in_=t["r128t"])
        shift_sb = consts.tile([P, 1], f32)
        nc.vector.memset(shift_sb[:], EXP_SHIFT)
        eps_sb = consts.tile([1, 1], f32)
        nc.vector.memset(eps_sb[:], EPS)

        # ================ Stage A: a-projections (kv strip first) ================
        with tc.tile_pool(name="a_sb", bufs=1) as a_sb_pool, \
             tc.tile_pool(name="a_wkv", bufs=16) as a_wkv_pool, \
             tc.tile_pool(name="a_wq", bufs=48) as a_wq_pool, \
             tc.tile_pool(name="a_ev", bufs=3) as a_ev_pool, \
             tc.tile_pool(name="a_ps", bufs=1, space="PSUM") as a_ps_pool, \
             tc.tile_pool(name="a_ssq", bufs=1, space="PSUM") as a_ssq_pool:
            hT_sb = []
            for qtr in range(4):
                ht = a_sb_pool.tile([P, 4, SC], f16, tag=f"hT{qtr}", name=f"hT{qtr}")
                dma(out=ht, in_=t["hT"][512 * qtr:512 * (qtr + 1), :]
                    .rearrange("(t p) s -> p t s", p=P))
                hT_sb.append(ht)

            def hts(k):
                return hT_sb[k // 4][:, k % 4, :]

            ssq_ps = [a_ssq_pool.tile([1, SC], f32, tag=f"ssq{i}", name=f"ssq{i}")
                      for i in range(2)]

            def evict(psum, dst, dst_row, ssq_grp, ssq_start, ssq_stop, rows=P):
                val = a_ev_pool.tile([P, SC], f16, tag="val")
                nc.vector.tensor_copy(val[0:rows, :], psum[0:rows, :])
                dma(out=dst[dst_row:dst_row + rows, :], in_=val[0:rows, :])
                if ssq_grp is not None:
                    sq = a_ev_pool.tile([P, SC], f16, tag="sq")
                    nc.scalar.activation(sq[:], psum[:], AF.Square)
                    MM(ssq_ps[ssq_grp][:], lhsT=ones_col_sb[:], rhs=sq[:],
                       start=ssq_start, stop=ssq_stop)

            # ---- kv strip (5 m-tiles: 4 c_kv + 1 rope, 640 cols) ----
            psums = [a_ps_pool.tile([P, SC], f32, tag=f"aps{i}", name=f"aps{i}")
                     for i in range(5)]
            for k in range(16):
                wt = a_wkv_pool.tile([P, 640], f16, tag="awkv")
                dma(out=wt, in_=t["kvawT_s"][k])
                for ml in range(5):
                    MM(psums[ml][:], lhsT=wt[:, ml * P:(ml + 1) * P],
                       rhs=hts(k), start=(k == 0), stop=(k == 15))
            for ml in range(4):
                evict(psums[ml], t["a_kv_loc"], ml * P, 1, ml == 0, ml == 3)
            evict(psums[4], t["a_kv_loc"], KVL, None, False, False, rows=D_ROPE)
            s_sb = a_ev_pool.tile([1, SC], f16, tag="ssqe")
            nc.vector.tensor_copy(s_sb[:], ssq_ps[1][:])
            dma(out=t["a_kv_loc"][KVL + D_ROPE:KVL + D_ROPE + 1, :], in_=s_sb[:])
            gather(t["a_kv_loc"], t["a_kv"])

            # ---- q strips (3 x 4 m-tiles); gathered in two pieces ----
            for mg in range(3):
                psums = [a_ps_pool.tile([P, SC], f32, tag=f"aps{i}", name=f"aps{i}")
                         for i in range(4)]
                for k in range(16):
                    wt = a_wq_pool.tile([P, 512], f16, tag="awq")
                    dma(out=wt, in_=t["qawT_s"][k, mg])
                    for ml in range(4):
                        MM(psums[ml][:], lhsT=wt[:, ml * P:(ml + 1) * P],
                           rhs=hts(k), start=(k == 0), stop=(k == 15))
                for ml in range(4):
                    m = mg * 4 + ml
                    if m < 6:
                        evict(psums[ml], t["a_q1_loc"], m * P, 0, m == 0, m == 11)
                    else:
                        evict(psums[ml], t["a_q2_loc"], (m - 6) * P, 0, m == 0, m == 11)
                if mg == 1:
                    gather(t["a_q1_loc"], t["a_q1"])
            s_sb = a_ev_pool.tile([1, SC], f16, tag="ssqe")
            nc.vector.tensor_copy(s_sb[:], ssq_ps[0][:])
            dma(out=t["a_q2_loc"][Q2_ROWS - 1:Q2_ROWS, :], in_=s_sb[:])
            gather(t["a_q2_loc"], t["a_q2"])

        # ================ persistent tiles + stage B/D weights ================
        qk_stack = ExitStack()
        pers_qk = qk_stack.enter_context(tc.tile_pool(name="pers_qk", bufs=1))
        pers_v = qk_stack.enter_context(tc.tile_pool(name="pers_v", bufs=1))
        q_npT = [pers_qk.tile([P, S], f16, tag=f"qnp{m}", name=f"qnp{m}") for m in range(NHL)]
        q_rpT = [pers_qk.tile([P, S], f16, tag=f"qrp{m}", name=f"qrp{m}") for m in range(2)]
        k_npT = [pers_qk.tile([P, S], f16, tag=f"knp{h}", name=f"knp{h}") for h in range(NHL)]
        k_rpT = pers_qk.tile([P, S], f16, tag="krp", name="krp")
        v_t = [pers_v.tile([P, NHL * D_V], f16, tag=f"v{st}", name=f"v{st}") for st in range(16)]
        pers_at = ec(tc.tile_pool(name="pers_at", bufs=1, side="right"))
        at_onT = [pers_at.tile([P, S], f16, tag=f"aon{h}", name=f"aon{h}") for h in range(NHL)]

        ow_sb = qk_stack.enter_context(tc.tile_pool(name="d_w", bufs=1)).tile([P, 4, H], f16)

        b_stack = ExitStack()
        bpool = b_stack.enter_context(tc.tile_pool(name="bpool", bufs=1))
        cosP_sb = bpool.tile([P, S], f32, tag="cosP")
        dma(out=cosP_sb, in_=t["cosP"])
        sinP_sb = bpool.tile([P, S], f32, tag="sinP")
        dma(out=sinP_sb, in_=t["sinP"])
        kvbw_k_sb = bpool.tile([P, 4, NHL * D_NOPE], f16, tag="kbk")
        dma(out=kvbw_k_sb, in_=t["kvbw_k"].rearrange("(t p) n -> p t n", p=P))
        kvbw_v_sb = bpool.tile([P, 4, NHL * D_V], f16, tag="kbv")
        dma(out=kvbw_v_sb, in_=t["kvbw_v"].rearrange("(t p) n -> p t n", p=P))
        qbw_np_sb = bpool.tile([P, 12, NHL * D_NOPE], f16, tag="qbn")
        dma(out=qbw_np_sb, in_=t["qbw_np"].rearrange("(t p) n -> p t n", p=P))
        qbw_rp_sb = bpool.tile([P, 12, NHL * D_ROPE], f16, tag="qbr")
        dma(out=qbw_rp_sb, in_=t["qbw_rp"].rearrange("(t p) n -> p t n", p=P))
        dma(out=ow_sb, in_=t["owT"].rearrange("(t p) n -> p t n", p=P))

        rdt = t["r_dram"].tensor
        bcast_rk = bpool.tile([P, S], f32, tag="brk")
        rkvT = bpool.tile([P, 16], f32, tag="rkvT")
        bcast_rq = bpool.tile([P, S], f32, tag="brq")

        # ---- rk scale (needs only the kv gather) ----
        # these DMAs wait on gather semaphores; issue them on the Activation
        # HWDGE queue so they don't head-of-line block the kv-pass loads on
        # the Sync queue
        sdma = nc.scalar.dma_start
        with tc.tile_pool(name="rtmp", bufs=1) as rtmp:
            ssq_kv = rtmp.tile([1, S], f16, tag="skv")
            sdma(out=ssq_kv.rearrange("r (c s) -> r c s", c=4),
                 in_=t["a_kv"][:, KVL + D_ROPE:KVL + D_ROPE + 1, :].rearrange("c r s -> r c s"))
            rs_kv = rtmp.tile([1, S], f32, tag="rskv")
            nc.scalar.activation(rs_kv[:], ssq_kv[:], AF.Sqrt,
                                 scale=1.0 / KVL, bias=eps_sb[:])
            rr_kv = rtmp.tile([1, S], f32, tag="rrkv")
            nc.vector.reciprocal(rr_kv[:], rs_kv[:])
            sdma(out=t["r_dram"][1:2, :], in_=rr_kv[:])
            sdma(out=bcast_rk, in_=bass.AP(tensor=rdt, offset=S, ap=[[0, P], [1, S]]))
            sdma(out=rkvT, in_=bass.AP(tensor=rdt, offset=S, ap=[[1, P], [P, 16]]))

        # ---- kv-pass: k/v up-projection (overlaps the q gathers) ----
        with tc.tile_pool(name="bk_s", bufs=3) as bk_s, \
             tc.tile_pool(name="bk_t", bufs=2) as bk_t, \
             tc.tile_pool(name="bk_ps", bufs=1, space="PSUM") as bk_ps, \
             tc.tile_pool(name="bk_ps2", bufs=2, space="PSUM") as bk_ps2:
            for c in range(4):
                cols = slice(c * SC, (c + 1) * SC)
                kva_sb = bk_s.tile([P, 4, SC], f16, tag="kva")
                for f4 in range(4):
                    dma(out=kva_sb[:, f4, :],
                        in_=t["a_kv"][c, f4 * P:(f4 + 1) * P, :])
                kps = [bk_ps.tile([P, SC], f32, tag=f"kps{h}", name=f"kps{h}")
                       for h in range(NHL)]
                for f in range(4):
                    for h in range(NHL):
                        MM(kps[h][:], lhsT=kvbw_k_sb[:, f, h * P:(h + 1) * P],
                           rhs=kva_sb[:, f, :], start=(f == 0), stop=(f == 3))
                for h in range(NHL):
                    nc.vector.tensor_mul(k_npT[h][:, cols], kps[h][:], bcast_rk[:, cols])
                for stl in range(4):
                    st = 4 * c + stl
                    vps = bk_ps2.tile([P, NHL * D_V], f32, tag="vps")
                    for f in range(4):
                        MM(vps[:], lhsT=kva_sb[:, f, stl * P:(stl + 1) * P],
                           rhs=kvbw_v_sb[:, f, :], start=(f == 0), stop=(f == 3))
                    nc.vector.tensor_scalar_mul(v_t[st][:], vps[:], rkvT[:, st:st + 1])
                # k_rope (loaded duplicated into both 64-partition halves)
                rp_f = bk_t.tile([P, SC], f16, tag="rp")
                kv_t_ = t["a_kv"].tensor
                rope_off = (c * KV_ROWS + KVL) * SC
                dma(out=rp_f[:],
                    in_=bass.AP(tensor=kv_t_, offset=rope_off,
                                ap=[[0, 2], [SC, D_ROPE], [1, SC]]))
                rot_ps = bk_ps2.tile([P, SC], f32, tag="rotk")
                MM(rot_ps[:], lhsT=r128_sb[:], rhs=rp_f[:], start=True, stop=True)
                t1 = bk_t.tile([P, SC], f32, tag="t1k")
                nc.vector.tensor_mul(t1[:], rot_ps[:], sinP_sb[:, cols])
                t2 = bk_t.tile([P, SC], f32, tag="t2k")
                nc.vector.tensor_mul(t2[:], rp_f[:], cosP_sb[:, cols])
                nc.vector.tensor_add(k_rpT[:, cols], t1[:], t2[:])

        # ---- rq scale (needs the second q gather) ----
        with tc.tile_pool(name="rtmp2", bufs=1) as rtmp2:
            ssq_q = rtmp2.tile([1, S], f16, tag="sq2")
            sdma(out=ssq_q.rearrange("r (c s) -> r c s", c=4),
                 in_=t["a_q2"][:, Q2_ROWS - 1:Q2_ROWS, :].rearrange("c r s -> r c s"))
            rs_q = rtmp2.tile([1, S], f32, tag="rsq2")
            nc.scalar.activation(rs_q[:], ssq_q[:], AF.Sqrt,
                                 scale=1.0 / QLR, bias=eps_sb[:])
            rr_q = rtmp2.tile([1, S], f32, tag="rrq2")
            nc.vector.reciprocal(rr_q[:], rs_q[:])
            sdma(out=t["r_dram"][0:1, :], in_=rr_q[:])
            sdma(out=bcast_rq, in_=bass.AP(tensor=rdt, offset=0, ap=[[0, P], [1, S]]))

        # ---- q-pass: q up-projection + rope for own heads ----
        # two 3-psum sets per chunk with double-buffered psums so chunk c+1's
        # matmuls overlap chunk c's evictions
        with tc.tile_pool(name="bq_s", bufs=2) as bq_s, \
             tc.tile_pool(name="bq_t", bufs=2) as bq_t, \
             tc.tile_pool(name="bq_ps", bufs=2, space="PSUM") as bq_ps, \
             tc.tile_pool(name="bq_ps2", bufs=2, space="PSUM") as bq_ps2:
            for c in range(4):
                cols = slice(c * SC, (c + 1) * SC)
                qa = []
                for f in range(12):
                    qa_f = bq_s.tile([P, SC], f16, tag=f"qa{f}")
                    if f < 6:
                        dma(out=qa_f, in_=t["a_q1"][c, f * P:(f + 1) * P, :])
                    else:
                        dma(out=qa_f, in_=t["a_q2"][c, (f - 6) * P:(f - 5) * P, :])
                    qa.append(qa_f)
                # set 0: np m-tiles 0..2
                qps = [bq_ps.tile([P, SC], f32, tag=f"qps{i}", name=f"qps{i}")
                       for i in range(3)]
                for f in range(12):
                    for i in range(3):
                        MM(qps[i][:], lhsT=qbw_np_sb[:, f, i * P:(i + 1) * P],
                           rhs=qa[f][:], start=(f == 0), stop=(f == 11))
                for i in range(3):
                    nc.vector.tensor_mul(q_npT[i][:, cols], qps[i][:], bcast_rq[:, cols])
                # set 1: np m-tile 3 + the two rope m-tiles
                qps = [bq_ps.tile([P, SC], f32, tag=f"qps{i}", name=f"qps{i}")
                       for i in range(3)]
                for f in range(12):
                    MM(qps[0][:], lhsT=qbw_np_sb[:, f, 3 * P:4 * P],
                       rhs=qa[f][:], start=(f == 0), stop=(f == 11))
                    for m2 in range(2):
                        MM(qps[1 + m2][:], lhsT=qbw_rp_sb[:, f, m2 * P:(m2 + 1) * P],
                           rhs=qa[f][:], start=(f == 0), stop=(f == 11))
                nc.vector.tensor_mul(q_npT[3][:, cols], qps[0][:], bcast_rq[:, cols])
                for m2 in range(2):
                    x_sb = bq_t.tile([P, SC], f16, tag="x")
                    nc.vector.tensor_mul(x_sb[:], qps[1 + m2][:], bcast_rq[:, cols])
                    rot_ps = bq_ps2.tile([P, SC], f32, tag="rot")
                    MM(rot_ps[:], lhsT=r128_sb[:], rhs=x_sb[:], start=True, stop=True)
                    t1 = bq_t.tile([P, SC], f32, tag="t1")
                    nc.vector.tensor_mul(t1[:], rot_ps[:], sinP_sb[:, cols])
                    t2 = bq_t.tile([P, SC], f32, tag="t2")
                    nc.vector.tensor_mul(t2[:], x_sb[:], cosP_sb[:, cols])
                    nc.vector.tensor_add(q_rpT[m2][:, cols], t1[:], t2[:])
        b_stack.close()

        # ================ attention + o-proj, interleaved per Q chunk ================
        c_stack = ExitStack()
        c_pt = c_stack.enter_context(tc.tile_pool(name="c_pt", bufs=4))
        c_da = c_stack.enter_context(tc.tile_pool(name="c_da", bufs=2))
        c_r = c_stack.enter_context(tc.tile_pool(name="c_r", bufs=2))
        d_o = c_stack.enter_context(tc.tile_pool(name="d_o", bufs=2))
        # shared [P,512] f32 PSUM pool: score tiles + o-proj accumulators
        c_sc = c_stack.enter_context(tc.tile_pool(name="c_sc", bufs=3, space="PSUM"))
        c_at = c_stack.enter_context(tc.tile_pool(name="c_at", bufs=3, space="PSUM"))
        c_dn = c_stack.enter_context(tc.tile_pool(name="c_dn", bufs=1, space="PSUM"))
        c_bc = c_stack.enter_context(tc.tile_pool(name="c_bc", bufs=1, space="PSUM"))

        def attn_head_chunk(h, Q):
            rp_tile = q_rpT[h // 2]
            rp_lo = D_ROPE * (h % 2)
            qcols = slice(Q * 512, (Q + 1) * 512)
            at_ps = c_at.tile([P, 512], f32, tag="at")
            dacc = c_da.tile([P, 512], f16, tag="dacc")
            jmax = 4 * Q + 3
            pend = []  # software pipeline: AV matmul for j lags scores by two
            for j in range(jmax + 1):
                jp = j - 4 * Q
                lo = max(jp, 0) * P
                qsl = slice(Q * 512 + lo, (Q + 1) * 512)
                ksl = slice(j * P, (j + 1) * P)
                sc_ps = c_sc.tile([P, 512], f32, tag="sc")
                MM(sc_ps[:, lo:], lhsT=k_npT[h][:, ksl], rhs=q_npT[h][:, qsl],
                   start=True, stop=False)
                MM(sc_ps[:, lo:], lhsT=k_rpT[rp_lo:rp_lo + D_ROPE, ksl],
                   rhs=rp_tile[rp_lo:rp_lo + D_ROPE, qsl],
                   start=False, stop=True)
                pt = c_pt.tile([P, 512], f16, tag="pt")
                nc.scalar.activation(pt[:, lo:], sc_ps[:, lo:], AF.Exp,
                                     scale=SCALING, bias=shift_sb[:])
                if jp >= 0:
                    nc.vector.tensor_mul(pt[:, lo:lo + P], pt[:, lo:lo + P],
                                         triu_sb[:])
                if j == 0:
                    nc.vector.tensor_copy(dacc[:], pt[:])
                else:
                    nc.vector.tensor_add(dacc[:, lo:], dacc[:, lo:], pt[:, lo:])
                pend.append((j, lo, pt))
                if len(pend) > 2:
                    pj, plo, ppt = pend.pop(0)
                    MM(at_ps[:, plo:], lhsT=v_t[pj][:, h * D_V:(h + 1) * D_V],
                       rhs=ppt[:, plo:], start=(pj == 0), stop=False)
            for pj, plo, ppt in pend:
                MM(at_ps[:, plo:], lhsT=v_t[pj][:, h * D_V:(h + 1) * D_V],
                   rhs=ppt[:, plo:], start=(pj == 0), stop=(pj == jmax))
            # denominator: partition-reduce the vector-accumulated dacc
            dn_ps = c_dn.tile([1, 512], f32, tag="dn")
            MM(dn_ps[:], lhsT=ones_col_sb[:], rhs=dacc[:], start=True, stop=True)
            rec = c_r.tile([1, 512], f32r, tag="rec")
            with nc.allow_low_precision(reason="f32r carries full fp32 bits"):
                nc.vector.reciprocal(rec[:], dn_ps[:])
            bc_ps = c_bc.tile([P, 512], f32, tag="bc")
            MM(bc_ps[:], lhsT=ones_row_sb[:], rhs=rec[:], start=True, stop=True)
            bc_sb = c_r.tile([P, 512], f32, tag="bcs")
            nc.vector.tensor_copy(bc_sb[:], bc_ps[:])
            nc.vector.tensor_mul(at_onT[h][:, qcols], at_ps[:], bc_sb[:])

        def oproj_chunk(Q):
            for qtl in range(4):
                qt = 4 * Q + qtl
                out_sb = d_o.tile([P, H], f16, tag="out")
                for hc in range(4):
                    psum = c_sc.tile([P, 512], f32, tag="sc")
                    for f in range(4):
                        MM(psum[:], lhsT=at_onT[f][:, qt * P:(qt + 1) * P],
                           rhs=ow_sb[:, f, hc * 512:(hc + 1) * 512],
                           start=(f == 0), stop=(f == 3))
                    nc.vector.tensor_copy(out_sb[:, hc * 512:(hc + 1) * 512], psum[:])
                dma(out=t["o_part"][qt * P:(qt + 1) * P, :], in_=out_sb[:])

        for Q in range(4):
            for h in range(NHL):
                attn_head_chunk(h, Q)
            oproj_chunk(Q)

        c_stack.close()
        qk_stack.close()


# ---------------- host side ----------------
_CACHED = {}


def _get_program():
    if "nc" not in _CACHED:
        _CACHED["nc"] = _build_program()
    return _CACHED["nc"]


def _host_consts():
    inv_freq = 1.0 / (ROPE_THETA ** (np.arange(0, D_ROPE, 2, dtype=np.float32) / D_ROPE))
    ti = np.arange(S, dtype=np.float32)
    ang = np.outer(ti, inv_freq)
    emb = np.concatenate([ang, ang], axis=-1)          # [S, 64]
    cosT = np.cos(emb).T.astype(np.float32)            # [64, S]
    sinT = np.sin(emb).T.astype(np.float32)
    cosP = np.vstack([cosT, cosT])                     # [128, S]
    sinP = np.vstack([sinT, sinT])
    r64 = np.zeros((D_ROPE, D_ROPE), np.float16)
    hlf = D_ROPE // 2
    for i in range(hlf):
        r64[i, i + hlf] = -1.0
        r64[i + hlf, i] = 1.0
    r128 = np.zeros((P, P), np.float16)
    r128[:D_ROPE, :D_ROPE] = r64
    r128[D_ROPE:, D_ROPE:] = r64
    r128t = np.ascontiguousarray(r128.T)
    kk, qq = np.meshgrid(np.arange(P), np.arange(P), indexing="ij")
    triu = (kk <= qq).astype(np.float16)
    return cosP, sinP, r128t, triu


def make_in_maps(hidden_states, q_a_w, q_a_ln_w, q_b_w, kv_a_w, kv_a_ln_w,
                 kv_b_w, o_w):
    f, f16_ = np.float32, np.float16
    hidden_states = np.asarray(hidden_states, f)
    q_b_eff = (np.asarray(q_b_w, f) * np.asarray(q_a_ln_w, f)[None, :]).astype(f16_)
    kv_b_eff = (np.asarray(kv_b_w, f) * np.asarray(kv_a_ln_w, f)[None, :]).astype(f16_)
    qawT = np.asarray(q_a_w, f).T.astype(f16_)         # [H, QLR]
    kvawT_pad = np.zeros((H, 5 * P), f16_)
    kvawT_pad[:, :KVL + D_ROPE] = np.asarray(kv_a_w, f).T.astype(f16_)
    qawT_s = np.ascontiguousarray(
        qawT.reshape(16, P, 3, 512).transpose(0, 2, 1, 3))
    kvawT_s = np.ascontiguousarray(kvawT_pad.reshape(16, P, 640))
    cosP, sinP, r128t, triu = _host_consts()
    ones_col = np.ones((P, 1), f16_)
    ones_row = np.ones((1, P), f)

    in_maps = []
    for core in range(N_CORES):
        b, g = divmod(core, 4)
        heads = range(NHL * g, NHL * (g + 1))
        hT = np.ascontiguousarray(hidden_states[b, g * SC:(g + 1) * SC, :].T.astype(f16_))
        qbw_np = np.ascontiguousarray(np.concatenate(
            [q_b_eff[D_QK * hh:D_QK * hh + D_NOPE] for hh in heads], 0).T)
        qbw_rp = np.ascontiguousarray(np.concatenate(
            [q_b_eff[D_QK * hh + D_NOPE:D_QK * (hh + 1)] for hh in heads], 0).T)
        kvbw_k = np.ascontiguousarray(np.concatenate(
            [kv_b_eff[(D_NOPE + D_V) * hh:(D_NOPE + D_V) * hh + D_NOPE]
             for hh in heads], 0).T)
        kvbw_v = np.ascontiguousarray(np.concatenate(
            [kv_b_eff[(D_NOPE + D_V) * hh + D_NOPE:(D_NOPE + D_V) * (hh + 1)]
             for hh in heads], 0).T)
        owT = np.ascontiguousarray(
            np.asarray(o_w, f)[:, g * NHL * D_V:(g + 1) * NHL * D_V].T.astype(f16_))
        in_maps.append(dict(
            hT=hT, qawT_s=qawT_s, kvawT_s=kvawT_s, qbw_np=qbw_np,
            qbw_rp=qbw_rp, kvbw_k=kvbw_k, kvbw_v=kvbw_v, owT=owT,
            cosP=cosP, sinP=sinP, r128t=r128t, triu=triu,
            ones_col=ones_col, ones_row=ones_row))
    return in_maps


def kernel(**inputs):
    nc = _get_program()
    in_maps = make_in_maps(**inputs)
    res = run_bass_kernel_spmd(nc, in_maps, core_ids=list(range(N_CORES)))
    out = np.zeros((B, S, H), np.float32)
    for core in range(N_CORES):
        out[core // 4] += res.results[core]["o_part"].astype(np.float32)
    return out


# revision 37
# speedup vs baseline: 1.1896x; 1.1896x over previous
"""DeepseekV3 MLA attention on 8 Trainium2 NeuronCores (Bass/Tile).

Sharding: core = (batch b, head-group g); b = core//4, g = core%4.
Each core handles 4 of 16 heads for one batch.

 - q_a / kv_a low-rank projections are sequence-sharded within each
   4-core batch group (each core computes a 512-column chunk of
   q_a^T / kv_a^T).  The kv strip is AllGathered first; the q strip is
   gathered in TWO pieces, each issued as soon as its rows are evicted,
   so all three collectives complete while the Tensor engine is still
   busy with the remaining strips and the kv up-projection — no
   exposed collective time.
 - rms scale for the kv path depends only on the kv gather, so the kv
   up-projection overlaps the q gathers.
 - q_b / kv_b up-projections + causal attention computed per-core for
   its 4 heads, everything in transposed (feature-on-partition) layout.
 - o-proj partial output per core ([S, H] fp16, contraction over the
   512 local v-dims); host sums the 4 partials per batch.

All matmul operands are fp16, accumulation fp32 in PSUM.  Softmax uses
exp(scale*s - 2) with no row-max subtraction (cancels in the ratio).
The softmax denominator is accumulated on the Vector engine (fp16 adds
of probability tiles) with one ones-vector matmul per (head, Q-chunk)
for the final partition reduction, and the attention inner loop is
software-pipelined (scores of block j+1 issue before the AV matmul of
block j) so the Tensor engine does not stall on the Exp activation.
o-proj is interleaved per Q-chunk right after its 4 heads finish.
"""
import sys

sys.path.insert(0, "/opt/trn_rl_repo")

import numpy as np

import concourse.bass as bass
import concourse.tile as tile
from concourse import bacc, mybir
from concourse.bass_utils import run_bass_kernel_spmd

# ---- problem constants (hardcoded per contract) ----
B, S, H = 2, 2048, 2048
NH, D_NOPE, D_ROPE, D_V = 16, 128, 64, 128
D_QK = D_NOPE + D_ROPE
QLR, KVL = 1536, 512
SCALING = float(D_QK) ** -0.5
EXP_SHIFT = -2.0   # global exponent shift; cancels in softmax ratio
EPS = 1e-6
ROPE_THETA = 10000.0

NHL = 4            # heads per core
N_CORES = 8
SC = S // 4        # seq chunk per core for the a-projections (512)
P = 128
KV_ROWS = KVL + D_ROPE + 1   # 577: c_kv rows, k_rope rows, kv sumsq row
Q1_ROWS = 1024               # first gathered q piece (f-tiles 0..7)
Q2_ROWS = QLR - Q1_ROWS + 1  # 513: f-tiles 8..11 plus the q sumsq row

f32 = mybir.dt.float32
f32r = mybir.dt.float32r
f16 = mybir.dt.float16
AF = mybir.ActivationFunctionType


def _build_program():
    nc = bacc.Bacc("TRN2", target_bir_lowering=False, debug=False,
                   num_devices=N_CORES)

    def din(name, shape, dt=f16):
        return nc.dram_tensor(name, list(shape), dt, kind="ExternalInput").ap()

    # per-core inputs (host-prepped layouts, fp16)
    hT = din("hT", [H, SC])
    qawT_s = din("qawT_s", [16, 3, P, 512])      # q_a_w.T strips [k, mg, 128, 512]
    kvawT_s = din("kvawT_s", [16, P, 640])       # kv_a_w.T strips (cols padded 576->640)
    qbw_np = din("qbw_np", [QLR, NHL * D_NOPE])  # [1536, 512] own heads
    qbw_rp = din("qbw_rp", [QLR, NHL * D_ROPE])  # [1536, 256] own heads
    kvbw_k = din("kvbw_k", [KVL, NHL * D_NOPE])  # [512, 512] own heads
    kvbw_v = din("kvbw_v", [KVL, NHL * D_V])     # [512, 512] own heads
    owT = din("owT", [NHL * D_V, H])             # [512, 2048] own heads
    cosP = din("cosP", [P, S], f32)              # vstack(cos.T, cos.T)
    sinP = din("sinP", [P, S], f32)
    r128t = din("r128t", [P, P])                 # rotate-half matrix (transposed, block-diag)
    triu = din("triu", [P, P])                   # causal mask for diagonal blocks
    ones_col = din("ones_col", [P, 1])           # lhsT for denominator matmul
    ones_row = din("ones_row", [1, P], f32)      # lhsT for partition-broadcast matmul (f32r)

    o_part = nc.dram_tensor("o_part", [S, H], f16, kind="ExternalOutput").ap()

    # scratch
    rec_dram = nc.dram_tensor("rec_dram", [16, 512], f32).ap()
    a_kv_loc = nc.dram_tensor("a_kv_loc", [KV_ROWS, SC], f16).ap()
    a_kv = nc.dram_tensor("a_kv", [4, KV_ROWS, SC], f16).ap()
    a_q1_loc = nc.dram_tensor("a_q1_loc", [Q1_ROWS, SC], f16).ap()
    a_q1 = nc.dram_tensor("a_q1", [4, Q1_ROWS, SC], f16).ap()
    a_q2_loc = nc.dram_tensor("a_q2_loc", [Q2_ROWS, SC], f16).ap()
    a_q2 = nc.dram_tensor("a_q2", [4, Q2_ROWS, SC], f16).ap()
    r_dram = nc.dram_tensor("r_dram", [2, S], f32).ap()

    with tile.TileContext(nc) as tc:
        _emit(tc, nc, dict(
            hT=hT, qawT_s=qawT_s, kvawT_s=kvawT_s, qbw_np=qbw_np,
            qbw_rp=qbw_rp, kvbw_k=kvbw_k, kvbw_v=kvbw_v, owT=owT,
            cosP=cosP, sinP=sinP, r128t=r128t, triu=triu,
            ones_col=ones_col, ones_row=ones_row,
            o_part=o_part, a_kv_loc=a_kv_loc, a_kv=a_kv,
            a_q1_loc=a_q1_loc, a_q1=a_q1, a_q2_loc=a_q2_loc, a_q2=a_q2,
            r_dram=r_dram, rec_dram=rec_dram))
    nc.compile()
    return nc


def _emit(tc, nc, t):
    from contextlib import ExitStack

    dma = nc.sync.dma_start
    MM = nc.tensor.matmul
    GROUPS = [[0, 1, 2, 3], [4, 5, 6, 7]]

    def gather(src, dst):
        nc.gpsimd.collective_compute(
            "AllGather", mybir.AluOpType.bypass, replica_groups=GROUPS,
            ins=[src.opt()], outs=[dst.opt()])

    with ExitStack() as stack:
        ec = stack.enter_context

        consts = ec(tc.tile_pool(name="consts", bufs=1))
        ones_col_sb = consts.tile([P, 1], f16)
        dma(out=ones_col_sb, in_=t["ones_col"])
        ones_row_sb = consts.tile([1, P], f32r)
        dma(out=ones_row_sb, in_=t["ones_row"].bitcast(f32r))
        triu_sb = consts.tile([P, P], f16)
        dma(out=triu_sb, in_=t["triu"])
        r128_sb = consts.tile([P, P], f16)
        dma(out=r128_sb, in_=t["r128t"])
        shift_sb = consts.tile([P, 1], f32)
        nc.vector.memset(shift_sb[:], EXP_SHIFT)
        eps_sb = consts.tile([P, 1], f32)
        nc.vector.memset(eps_sb[:], EPS)

        # ================ Stage A: a-projections (kv strip first) ================
        with tc.tile_pool(name="a_sb", bufs=1) as a_sb_pool, \
             tc.tile_pool(name="a_wkv", bufs=16) as a_wkv_pool, \
             tc.tile_pool(name="a_wq", bufs=48) as a_wq_pool, \
             tc.tile_pool(name="a_ev", bufs=3) as a_ev_pool, \
             tc.tile_pool(name="a_ps", bufs=1, space="PSUM") as a_ps_pool, \
             tc.tile_pool(name="a_ssq", bufs=1, space="PSUM") as a_ssq_pool:
            hT_sb = []
            for qtr in range(4):
                ht = a_sb_pool.tile([P, 4, SC], f16, tag=f"hT{qtr}", name=f"hT{qtr}")
                dma(out=ht, in_=t["hT"][512 * qtr:512 * (qtr + 1), :]
                    .rearrange("(t p) s -> p t s", p=P))
                hT_sb.append(ht)

            def hts(k):
                return hT_sb[k // 4][:, k % 4, :]

            ssq_ps = [a_ssq_pool.tile([1, SC], f32, tag=f"ssq{i}", name=f"ssq{i}")
                      for i in range(2)]

            def evict(psum, dst, dst_row, ssq_grp, ssq_start, ssq_stop, rows=P):
                val = a_ev_pool.tile([P, SC], f16, tag="val")
                nc.vector.tensor_copy(val[0:rows, :], psum[0:rows, :])
                dma(out=dst[dst_row:dst_row + rows, :], in_=val[0:rows, :])
                if ssq_grp is not None:
                    sq = a_ev_pool.tile([P, SC], f16, tag="sq")
                    nc.scalar.activation(sq[:], psum[:], AF.Square)
                    MM(ssq_ps[ssq_grp][:], lhsT=ones_col_sb[:], rhs=sq[:],
                       start=ssq_start, stop=ssq_stop)

            # ---- kv strip (5 m-tiles: 4 c_kv + 1 rope, 640 cols) ----
            psums = [a_ps_pool.tile([P, SC], f32, tag=f"aps{i}", name=f"aps{i}")
                     for i in range(5)]
            for k in range(16):
                wt = a_wkv_pool.tile([P, 640], f16, tag="awkv")
                dma(out=wt, in_=t["kvawT_s"][k])
                for ml in range(5):
                    MM(psums[ml][:], lhsT=wt[:, ml * P:(ml + 1) * P],
                       rhs=hts(k), start=(k == 0), stop=(k == 15))
            for ml in range(4):
                evict(psums[ml], t["a_kv_loc"], ml * P, 1, ml == 0, ml == 3)
            evict(psums[4], t["a_kv_loc"], KVL, None, False, False, rows=D_ROPE)
            s_sb = a_ev_pool.tile([1, SC], f16, tag="ssqe")
            nc.vector.tensor_copy(s_sb[:], ssq_ps[1][:])
            dma(out=t["a_kv_loc"][KVL + D_ROPE:KVL + D_ROPE + 1, :], in_=s_sb[:])
            gather(t["a_kv_loc"], t["a_kv"])

            # ---- q strips (3 x 4 m-tiles); gathered in two pieces ----
            for mg in range(3):
                psums = [a_ps_pool.tile([P, SC], f32, tag=f"aps{i}", name=f"aps{i}")
                         for i in range(4)]
                for k in range(16):
                    wt = a_wq_pool.tile([P, 512], f16, tag="awq")
                    dma(out=wt, in_=t["qawT_s"][k, mg])
                    for ml in range(4):
                        MM(psums[ml][:], lhsT=wt[:, ml * P:(ml + 1) * P],
                           rhs=hts(k), start=(k == 0), stop=(k == 15))
                for ml in range(4):
                    m = mg * 4 + ml
                    if m < 8:
                        evict(psums[ml], t["a_q1_loc"], m * P, 0, m == 0, m == 11)
                    else:
                        evict(psums[ml], t["a_q2_loc"], (m - 8) * P, 0, m == 0, m == 11)
                if mg == 1:
                    gather(t["a_q1_loc"], t["a_q1"])
            s_sb = a_ev_pool.tile([1, SC], f16, tag="ssqe")
            nc.vector.tensor_copy(s_sb[:], ssq_ps[0][:])
            dma(out=t["a_q2_loc"][Q2_ROWS - 1:Q2_ROWS, :], in_=s_sb[:])
            gather(t["a_q2_loc"], t["a_q2"])

        # ================ persistent tiles + stage B/D weights ================
        qk_stack = ExitStack()
        pers_qk = qk_stack.enter_context(tc.tile_pool(name="pers_qk", bufs=1))
        pers_v = qk_stack.enter_context(tc.tile_pool(name="pers_v", bufs=1))
        q_npT = [pers_qk.tile([P, S], f16, tag=f"qnp{m}", name=f"qnp{m}") for m in range(NHL)]
        q_rpT = [pers_qk.tile([P, S], f16, tag=f"qrp{m}", name=f"qrp{m}") for m in range(2)]
        k_npT = [pers_qk.tile([P, S], f16, tag=f"knp{h}", name=f"knp{h}") for h in range(NHL)]
        k_rpT = pers_qk.tile([P, S], f16, tag="krp", name="krp")
        v_t = [pers_v.tile([P, NHL * D_V], f16, tag=f"v{st}", name=f"v{st}") for st in range(16)]
        pers_at = ec(tc.tile_pool(name="pers_at", bufs=1, side="right"))
        at_onT = [pers_at.tile([P, S], f16, tag=f"aon{h}", name=f"aon{h}") for h in range(NHL)]

        ow_sb = qk_stack.enter_context(tc.tile_pool(name="d_w", bufs=1)).tile([P, 4, H], f16)

        b_stack = ExitStack()
        bpool = b_stack.enter_context(tc.tile_pool(name="bpool", bufs=1))
        cosP_sb = bpool.tile([P, S], f32, tag="cosP")
        dma(out=cosP_sb, in_=t["cosP"])
        sinP_sb = bpool.tile([P, S], f32, tag="sinP")
        dma(out=sinP_sb, in_=t["sinP"])
        kvbw_k_sb = bpool.tile([P, 4, NHL * D_NOPE], f16, tag="kbk")
        dma(out=kvbw_k_sb, in_=t["kvbw_k"].rearrange("(t p) n -> p t n", p=P))
        kvbw_v_sb = bpool.tile([P, 4, NHL * D_V], f16, tag="kbv")
        dma(out=kvbw_v_sb, in_=t["kvbw_v"].rearrange("(t p) n -> p t n", p=P))
        qbw_np_sb = bpool.tile([P, 12, NHL * D_NOPE], f16, tag="qbn")
        dma(out=qbw_np_sb, in_=t["qbw_np"].rearrange("(t p) n -> p t n", p=P))
        qbw_rp_sb = bpool.tile([P, 12, NHL * D_ROPE], f16, tag="qbr")
        dma(out=qbw_rp_sb, in_=t["qbw_rp"].rearrange("(t p) n -> p t n", p=P))
        dma(out=ow_sb, in_=t["owT"].rearrange("(t p) n -> p t n", p=P))

        rdt = t["r_dram"].tensor
        bcast_rk = bpool.tile([P, S], f32, tag="brk")
        rkvT = bpool.tile([P, 16], f32, tag="rkvT")
        bcast_rq = bpool.tile([P, S], f32, tag="brq")

        # ---- rk scale (needs only the kv gather) ----
        # [4,512] shapes (chunk on partitions) keep the DVE reciprocal fast;
        # DMAs go on the Activation HWDGE queue so they don't head-of-line
        # block the kv-pass loads on the Sync queue; tiles live in bpool so
        # no pool-reuse dependencies are created
        sdma = nc.scalar.dma_start
        ssq_kv = bpool.tile([4, SC], f16, tag="skv")
        sdma(out=ssq_kv,
             in_=t["a_kv"][:, KVL + D_ROPE:KVL + D_ROPE + 1, :].rearrange("c r s -> (c r) s"))
        rs_kv = bpool.tile([4, SC], f32, tag="rskv")
        nc.scalar.activation(rs_kv[:], ssq_kv[:], AF.Sqrt,
                             scale=1.0 / KVL, bias=eps_sb[0:4, :])
        rr_kv = bpool.tile([4, SC], f32, tag="rrkv")
        nc.vector.reciprocal_approx_fast(rr_kv[:], rs_kv[:])
        sdma(out=bass.AP(tensor=rdt, offset=S, ap=[[512, 4], [1, SC]]), in_=rr_kv[:])
        sdma(out=bcast_rk, in_=bass.AP(tensor=rdt, offset=S, ap=[[0, P], [1, S]]))
        sdma(out=rkvT, in_=bass.AP(tensor=rdt, offset=S, ap=[[1, P], [P, 16]]))

        # ---- kv-pass: k/v up-projection (overlaps the q gathers) ----
        with tc.tile_pool(name="bk_s", bufs=3) as bk_s, \
             tc.tile_pool(name="bk_t", bufs=2) as bk_t, \
             tc.tile_pool(name="bk_ps", bufs=1, space="PSUM") as bk_ps, \
             tc.tile_pool(name="bk_ps2", bufs=2, space="PSUM") as bk_ps2:
            for c in range(4):
                cols = slice(c * SC, (c + 1) * SC)
                kva_sb = bk_s.tile([P, 4, SC], f16, tag="kva")
                for f4 in range(4):
                    dma(out=kva_sb[:, f4, :],
                        in_=t["a_kv"][c, f4 * P:(f4 + 1) * P, :])
                kps = [bk_ps.tile([P, SC], f32, tag=f"kps{h}", name=f"kps{h}")
                       for h in range(NHL)]
                for f in range(4):
                    for h in range(NHL):
                        MM(kps[h][:], lhsT=kvbw_k_sb[:, f, h * P:(h + 1) * P],
                           rhs=kva_sb[:, f, :], start=(f == 0), stop=(f == 3))
                for h in range(NHL):
                    nc.vector.tensor_mul(k_npT[h][:, cols], kps[h][:], bcast_rk[:, cols])
                for stl in range(4):
                    st = 4 * c + stl
                    vps = bk_ps2.tile([P, NHL * D_V], f32, tag="vps")
                    for f in range(4):
                        MM(vps[:], lhsT=kva_sb[:, f, stl * P:(stl + 1) * P],
                           rhs=kvbw_v_sb[:, f, :], start=(f == 0), stop=(f == 3))
                    nc.vector.tensor_scalar_mul(v_t[st][:], vps[:], rkvT[:, st:st + 1])
                # k_rope (loaded duplicated into both 64-partition halves)
                rp_f = bk_t.tile([P, SC], f16, tag="rp")
                kv_t_ = t["a_kv"].tensor
                rope_off = (c * KV_ROWS + KVL) * SC
                dma(out=rp_f[:],
                    in_=bass.AP(tensor=kv_t_, offset=rope_off,
                                ap=[[0, 2], [SC, D_ROPE], [1, SC]]))
                rot_ps = bk_ps2.tile([P, SC], f32, tag="rotk")
                MM(rot_ps[:], lhsT=r128_sb[:], rhs=rp_f[:], start=True, stop=True)
                t1 = bk_t.tile([P, SC], f32, tag="t1k")
                nc.vector.tensor_mul(t1[:], rot_ps[:], sinP_sb[:, cols])
                t2 = bk_t.tile([P, SC], f32, tag="t2k")
                nc.vector.tensor_mul(t2[:], rp_f[:], cosP_sb[:, cols])
                nc.vector.tensor_add(k_rpT[:, cols], t1[:], t2[:])

        # ---- rq scale (needs the second q gather) ----
        ssq_q = bpool.tile([4, SC], f16, tag="sq2")
        sdma(out=ssq_q,
             in_=t["a_q2"][:, Q2_ROWS - 1:Q2_ROWS, :].rearrange("c r s -> (c r) s"))
        rs_q = bpool.tile([4, SC], f32, tag="rsq2")
        nc.scalar.activation(rs_q[:], ssq_q[:], AF.Sqrt,
                             scale=1.0 / QLR, bias=eps_sb[0:4, :])
        rr_q = bpool.tile([4, SC], f32, tag="rrq2")
        nc.vector.reciprocal_approx_fast(rr_q[:], rs_q[:])
        sdma(out=bass.AP(tensor=rdt, offset=0, ap=[[512, 4], [1, SC]]), in_=rr_q[:])
        sdma(out=bcast_rq, in_=bass.AP(tensor=rdt, offset=0, ap=[[0, P], [1, S]]))

        # ---- q-pass: q up-projection + rope for own heads ----
        # f ascends so each chunk's f<8 matmuls can start as soon as the
        # first q gather lands, before the second gather completes
        with tc.tile_pool(name="bq_s", bufs=6) as bq_s, \
             tc.tile_pool(name="bq_t", bufs=2) as bq_t, \
             tc.tile_pool(name="bq_ps", bufs=1, space="PSUM") as bq_ps, \
             tc.tile_pool(name="bq_ps2", bufs=2, space="PSUM") as bq_ps2:
            for c in range(4):
                cols = slice(c * SC, (c + 1) * SC)
                qps = [bq_ps.tile([P, SC], f32, tag=f"qps{m}", name=f"qps{m}")
                       for m in range(6)]
                for f in range(12):
                    qa_f = bq_s.tile([P, SC], f16, tag="qa")
                    if f < 8:
                        dma(out=qa_f, in_=t["a_q1"][c, f * P:(f + 1) * P, :])
                    else:
                        dma(out=qa_f, in_=t["a_q2"][c, (f - 8) * P:(f - 7) * P, :])
                    for m in range(4):
                        MM(qps[m][:], lhsT=qbw_np_sb[:, f, m * P:(m + 1) * P],
                           rhs=qa_f[:], start=(f == 0), stop=(f == 11))
                    for m2 in range(2):
                        MM(qps[4 + m2][:], lhsT=qbw_rp_sb[:, f, m2 * P:(m2 + 1) * P],
                           rhs=qa_f[:], start=(f == 0), stop=(f == 11))
                for m in range(4):
                    nc.vector.tensor_mul(q_npT[m][:, cols], qps[m][:], bcast_rq[:, cols])
                for m2 in range(2):
                    x_sb = bq_t.tile([P, SC], f16, tag="x")
                    nc.vector.tensor_mul(x_sb[:], qps[4 + m2][:], bcast_rq[:, cols])
                    rot_ps = bq_ps2.tile([P, SC], f32, tag="rot")
                    MM(rot_ps[:], lhsT=r128_sb[:], rhs=x_sb[:], start=True, stop=True)
                    t1 = bq_t.tile([P, SC], f32, tag="t1")
                    nc.vector.tensor_mul(t1[:], rot_ps[:], sinP_sb[:, cols])
                    t2 = bq_t.tile([P, SC], f32, tag="t2")
                    nc.vector.tensor_mul(t2[:], x_sb[:], cosP_sb[:, cols])
                    nc.vector.tensor_add(q_rpT[m2][:, cols], t1[:], t2[:])
        b_stack.close()

        # ================ attention + o-proj, interleaved per Q chunk ================
        c_stack = ExitStack()
        c_pt = c_stack.enter_context(tc.tile_pool(name="c_pt", bufs=6))
        c_da = c_stack.enter_context(tc.tile_pool(name="c_da", bufs=2))
        c_r = c_stack.enter_context(tc.tile_pool(name="c_r", bufs=2))
        d_o = c_stack.enter_context(tc.tile_pool(name="d_o", bufs=2))
        # shared [P,512] f32 PSUM pool: score tiles + o-proj accumulators
        c_sc = c_stack.enter_context(tc.tile_pool(name="c_sc", bufs=4, space="PSUM"))
        c_at = c_stack.enter_context(tc.tile_pool(name="c_at", bufs=3, space="PSUM"))
        c_dn = c_stack.enter_context(tc.tile_pool(name="c_dn", bufs=1, space="PSUM"))
        rec_t = t["rec_dram"].tensor

        def attn_head_chunk(h, Q):
            rp_tile = q_rpT[h // 2]
            rp_lo = D_ROPE * (h % 2)
            qcols = slice(Q * 512, (Q + 1) * 512)
            at_ps = c_at.tile([P, 512], f32, tag="at")
            dacc = c_da.tile([P, 512], f16, tag="dacc")
            jmax = 4 * Q + 3
            pend = []  # software pipeline: AV matmul for j lags scores by two
            for j in range(jmax + 1):
                jp = j - 4 * Q
                lo = max(jp, 0) * P
                qsl = slice(Q * 512 + lo, (Q + 1) * 512)
                ksl = slice(j * P, (j + 1) * P)
                sc_ps = c_sc.tile([P, 512], f32, tag="sc")
                MM(sc_ps[:, lo:], lhsT=k_npT[h][:, ksl], rhs=q_npT[h][:, qsl],
                   start=True, stop=False)
                MM(sc_ps[:, lo:], lhsT=k_rpT[rp_lo:rp_lo + D_ROPE, ksl],
                   rhs=rp_tile[rp_lo:rp_lo + D_ROPE, qsl],
                   start=False, stop=True)
                pt = c_pt.tile([P, 512], f16, tag="pt")
                nc.scalar.activation(pt[:, lo:], sc_ps[:, lo:], AF.Exp,
                                     scale=SCALING, bias=shift_sb[:])
                if jp >= 0:
                    nc.vector.tensor_mul(pt[:, lo:lo + P], pt[:, lo:lo + P],
                                         triu_sb[:])
                if j == 0:
                    nc.vector.tensor_copy(dacc[:], pt[:])
                else:
                    nc.vector.tensor_add(dacc[:, lo:], dacc[:, lo:], pt[:, lo:])
                pend.append((j, lo, pt))
                if len(pend) > 2:
                    pj, plo, ppt = pend.pop(0)
                    MM(at_ps[:, plo:], lhsT=v_t[pj][:, h * D_V:(h + 1) * D_V],
                       rhs=ppt[:, plo:], start=(pj == 0), stop=False)
            for pj, plo, ppt in pend:
                MM(at_ps[:, plo:], lhsT=v_t[pj][:, h * D_V:(h + 1) * D_V],
                   rhs=ppt[:, plo:], start=(pj == 0), stop=(pj == jmax))
            # denominator: partition-reduce the vector-accumulated dacc, then
            # broadcast 1/dn across partitions via a DRAM round-trip (stride-0
            # partition read) so the Tensor queue never stalls on this tail
            dn_ps = c_dn.tile([1, 512], f32, tag="dn")
            MM(dn_ps[:], lhsT=ones_col_sb[:], rhs=dacc[:], start=True, stop=True)
            rec = c_r.tile([1, 512], f32, tag="rec")
            nc.vector.reciprocal_approx_fast(rec[:], dn_ps[:])
            hq = h * 4 + Q
            dma(out=t["rec_dram"][hq:hq + 1, :], in_=rec[:])
            bc_sb = c_r.tile([P, 512], f32, tag="bcs")
            dma(out=bc_sb, in_=bass.AP(tensor=rec_t, offset=hq * 512,
                                       ap=[[0, P], [1, 512]]))
            nc.vector.tensor_mul(at_onT[h][:, qcols], at_ps[:], bc_sb[:])

        def oproj_chunk(Q):
            for qtl in range(4):
                qt = 4 * Q + qtl
                out_sb = d_o.tile([P, H], f16, tag="out")
                for hc in range(4):
                    psum = c_sc.tile([P, 512], f32, tag="sc")
                    for f in range(4):
                        MM(psum[:], lhsT=at_onT[f][:, qt * P:(qt + 1) * P],
                           rhs=ow_sb[:, f, hc * 512:(hc + 1) * 512],
                           start=(f == 0), stop=(f == 3))
                    nc.vector.tensor_copy(out_sb[:, hc * 512:(hc + 1) * 512], psum[:])
                dma(out=t["o_part"][qt * P:(qt + 1) * P, :], in_=out_sb[:])

        for Q in range(4):
            for h in range(NHL):
                attn_head_chunk(h, Q)
            oproj_chunk(Q)

        c_stack.close()
        qk_stack.close()


# ---------------- host side ----------------
_CACHED = {}


def _get_program():
    if "nc" not in _CACHED:
        _CACHED["nc"] = _build_program()
    return _CACHED["nc"]


def _host_consts():
    inv_freq = 1.0 / (ROPE_THETA ** (np.arange(0, D_ROPE, 2, dtype=np.float32) / D_ROPE))
    ti = np.arange(S, dtype=np.float32)
    ang = np.outer(ti, inv_freq)
    emb = np.concatenate([ang, ang], axis=-1)          # [S, 64]
    cosT = np.cos(emb).T.astype(np.float32)            # [64, S]
    sinT = np.sin(emb).T.astype(np.float32)
    cosP = np.vstack([cosT, cosT])                     # [128, S]
    sinP = np.vstack([sinT, sinT])
    r64 = np.zeros((D_ROPE, D_ROPE), np.float16)
    hlf = D_ROPE // 2
    for i in range(hlf):
        r64[i, i + hlf] = -1.0
        r64[i + hlf, i] = 1.0
    r128 = np.zeros((P, P), np.float16)
    r128[:D_ROPE, :D_ROPE] = r64
    r128[D_ROPE:, D_ROPE:] = r64
    r128t = np.ascontiguousarray(r128.T)
    kk, qq = np.meshgrid(np.arange(P), np.arange(P), indexing="ij")
    triu = (kk <= qq).astype(np.float16)
    return cosP, sinP, r128t, triu


def make_in_maps(hidden_states, q_a_w, q_a_ln_w, q_b_w, kv_a_w, kv_a_ln_w,
                 kv_b_w, o_w):
    f, f16_ = np.float32, np.float16
    hidden_states = np.asarray(hidden_states, f)
    q_b_eff = (np.asarray(q_b_w, f) * np.asarray(q_a_ln_w, f)[None, :]).astype(f16_)
    kv_b_eff = (np.asarray(kv_b_w, f) * np.asarray(kv_a_ln_w, f)[None, :]).astype(f16_)
    qawT = np.asarray(q_a_w, f).T.astype(f16_)         # [H, QLR]
    kvawT_pad = np.zeros((H, 5 * P), f16_)
    kvawT_pad[:, :KVL + D_ROPE] = np.asarray(kv_a_w, f).T.astype(f16_)
    qawT_s = np.ascontiguousarray(
        qawT.reshape(16, P, 3, 512).transpose(0, 2, 1, 3))
    kvawT_s = np.ascontiguousarray(kvawT_pad.reshape(16, P, 640))
    cosP, sinP, r128t, triu = _host_consts()
    ones_col = np.ones((P, 1), f16_)
    ones_row = np.ones((1, P), f)

    in_maps = []
    for core in range(N_CORES):
        b, g = divmod(core, 4)
        heads = range(NHL * g, NHL * (g + 1))
        hT = np.ascontiguousarray(hidden_states[b, g * SC:(g + 1) * SC, :].T.astype(f16_))
        qbw_np = np.ascontiguousarray(np.concatenate(
            [q_b_eff[D_QK * hh:D_QK * hh + D_NOPE] for hh in heads], 0).T)
        qbw_rp = np.ascontiguousarray(np.concatenate(
            [q_b_eff[D_QK * hh + D_NOPE:D_QK * (hh + 1)] for hh in heads], 0).T)
        kvbw_k = np.ascontiguousarray(np.concatenate(
            [kv_b_eff[(D_NOPE + D_V) * hh:(D_NOPE + D_V) * hh + D_NOPE]
             for hh in heads], 0).T)
        kvbw_v = np.ascontiguousarray(np.concatenate(
            [kv_b_eff[(D_NOPE + D_V) * hh + D_NOPE:(D_NOPE + D_V) * (hh + 1)]
             for hh in heads], 0).T)
        owT = np.ascontiguousarray(
            np.asarray(o_w, f)[:, g * NHL * D_V:(g + 1) * NHL * D_V].T.astype(f16_))
        in_maps.append(dict(
            hT=hT, qawT_s=qawT_s, kvawT_s=kvawT_s, qbw_np=qbw_np,
            qbw_rp=qbw_rp, kvbw_k=kvbw_k, kvbw_v=kvbw_v, owT=owT,
            cosP=cosP, sinP=sinP, r128t=r128t, triu=triu,
            ones_col=ones_col, ones_row=ones_row))
    return in_maps


def kernel(**inputs):
    nc = _get_program()
    in_maps = make_in_maps(**inputs)
    res = run_bass_kernel_spmd(nc, in_maps, core_ids=list(range(N_CORES)))
    out = np.zeros((B, S, H), np.float32)
    for core in range(N_CORES):
        out[core // 4] += res.results[core]["o_part"].astype(np.float32)
    return out


# revision 46
# speedup vs baseline: 1.2916x; 1.0857x over previous
"""DeepseekV3 MLA attention on 8 Trainium2 NeuronCores (Bass/Tile).

Sharding: core = (batch b, head-group g); b = core//4, g = core%4.
Each core handles 4 of 16 heads for one batch.

 - q_a / kv_a low-rank projections are sequence-sharded within each
   4-core batch group (each core computes a 512-column chunk of
   q_a^T / kv_a^T).  The kv strip is AllGathered first; the q strip is
   gathered in TWO pieces, each issued as soon as its rows are evicted,
   so all three collectives complete while the Tensor engine is still
   busy with the remaining strips and the kv up-projection — no
   exposed collective time.
 - rms scale for the kv path depends only on the kv gather, so the kv
   up-projection overlaps the q gathers.
 - q_b / kv_b up-projections + causal attention computed per-core for
   its 4 heads, everything in transposed (feature-on-partition) layout.
 - o-proj partial output per core ([S, H] fp16, contraction over the
   512 local v-dims); host sums the 4 partials per batch.

All matmul operands are fp16, accumulation fp32 in PSUM.  Softmax uses
exp(scale*s - 2) with no row-max subtraction (cancels in the ratio).
The softmax denominator is accumulated on the Vector engine (fp16 adds
of probability tiles) with one ones-vector matmul per (head, Q-chunk)
for the final partition reduction, and the attention inner loop is
software-pipelined (scores of block j+1 issue before the AV matmul of
block j) so the Tensor engine does not stall on the Exp activation.
o-proj is interleaved per Q-chunk right after its 4 heads finish.
"""
import sys

sys.path.insert(0, "/opt/trn_rl_repo")

import numpy as np

import concourse.bass as bass
import concourse.tile as tile
from concourse import bacc, mybir
from concourse.bass_utils import run_bass_kernel_spmd

# ---- problem constants (hardcoded per contract) ----
B, S, H = 2, 2048, 2048
NH, D_NOPE, D_ROPE, D_V = 16, 128, 64, 128
D_QK = D_NOPE + D_ROPE
QLR, KVL = 1536, 512
SCALING = float(D_QK) ** -0.5
EXP_SHIFT = -2.0   # global exponent shift; cancels in softmax ratio
EPS = 1e-6
ROPE_THETA = 10000.0

NHL = 4            # heads per core
N_CORES = 8
SC = S // 4        # seq chunk per core for the a-projections (512)
P = 128
KV_ROWS = KVL + D_ROPE + 1   # 577: c_kv rows, k_rope rows, kv sumsq row
# q_a is gathered in fp8-e3m4 (values are ~N(0,1); e3m4 spans +-15.5 at
# ~1.5-3% relative error, and the rel-err budget is 2e-2) which keeps the
# whole 1536-row piece under the 1 MB Mesh AllGather limit in ONE gather;
# the f16 sumsq row rides a separate tiny gather

f32 = mybir.dt.float32
f32r = mybir.dt.float32r
f16 = mybir.dt.float16
f8 = mybir.dt.float8e3
AF = mybir.ActivationFunctionType


def _build_program():
    nc = bacc.Bacc("TRN2", target_bir_lowering=False, debug=False,
                   num_devices=N_CORES)

    def din(name, shape, dt=f16):
        return nc.dram_tensor(name, list(shape), dt, kind="ExternalInput").ap()

    # per-core inputs (host-prepped layouts, fp16)
    hT = din("hT", [H, SC])
    qawT_s = din("qawT_s", [16, 3, P, 512])      # q_a_w.T strips [k, mg, 128, 512]
    kvawT_s = din("kvawT_s", [16, P, 640])       # kv_a_w.T strips (cols padded 576->640)
    qbw_np = din("qbw_np", [QLR, NHL * D_NOPE])  # [1536, 512] own heads
    qbw_rp = din("qbw_rp", [QLR, NHL * D_ROPE])  # [1536, 256] own heads
    kvbw_k = din("kvbw_k", [KVL, NHL * D_NOPE])  # [512, 512] own heads
    kvbw_v = din("kvbw_v", [KVL, NHL * D_V])     # [512, 512] own heads
    owT = din("owT", [NHL * D_V, H])             # [512, 2048] own heads
    cosP = din("cosP", [P, S], f32)              # vstack(cos.T, cos.T)
    sinP = din("sinP", [P, S], f32)
    r128t = din("r128t", [P, P])                 # rotate-half matrix (transposed, block-diag)
    triu = din("triu", [P, P])                   # causal mask for diagonal blocks
    ones_col = din("ones_col", [P, 1])           # lhsT for denominator matmul
    ones_row = din("ones_row", [1, P], f32)      # lhsT for partition-broadcast matmul (f32r)

    o_part = nc.dram_tensor("o_part", [S, H], f16, kind="ExternalOutput").ap()

    # scratch
    rec_dram = nc.dram_tensor("rec_dram", [16, 512], f32).ap()
    a_kv_loc = nc.dram_tensor("a_kv_loc", [KV_ROWS, SC], f16).ap()
    a_kv = nc.dram_tensor("a_kv", [4, KV_ROWS, SC], f16).ap()
    a_q_loc = nc.dram_tensor("a_q_loc", [QLR, SC], f8).ap()
    a_q = nc.dram_tensor("a_q", [4, QLR, SC], f8).ap()
    a_qs_loc = nc.dram_tensor("a_qs_loc", [1, SC], f16).ap()
    a_qs = nc.dram_tensor("a_qs", [4, 1, SC], f16).ap()
    r_dram = nc.dram_tensor("r_dram", [2, S], f32).ap()

    with tile.TileContext(nc) as tc:
        _emit(tc, nc, dict(
            hT=hT, qawT_s=qawT_s, kvawT_s=kvawT_s, qbw_np=qbw_np,
            qbw_rp=qbw_rp, kvbw_k=kvbw_k, kvbw_v=kvbw_v, owT=owT,
            cosP=cosP, sinP=sinP, r128t=r128t, triu=triu,
            ones_col=ones_col, ones_row=ones_row,
            o_part=o_part, a_kv_loc=a_kv_loc, a_kv=a_kv,
            a_q_loc=a_q_loc, a_q=a_q, a_qs_loc=a_qs_loc, a_qs=a_qs,
            r_dram=r_dram, rec_dram=rec_dram))
    nc.compile()
    return nc


def _emit(tc, nc, t):
    from contextlib import ExitStack

    dma = nc.sync.dma_start
    MM = nc.tensor.matmul
    GROUPS = [[0, 1, 2, 3], [4, 5, 6, 7]]

    def gather(src, dst):
        nc.gpsimd.collective_compute(
            "AllGather", mybir.AluOpType.bypass, replica_groups=GROUPS,
            ins=[src.opt()], outs=[dst.opt()])

    with ExitStack() as stack:
        ec = stack.enter_context

        consts = ec(tc.tile_pool(name="consts", bufs=1))
        ones_col_sb = consts.tile([P, 1], f16)
        dma(out=ones_col_sb, in_=t["ones_col"])
        ones_row_sb = consts.tile([1, P], f32r)
        dma(out=ones_row_sb, in_=t["ones_row"].bitcast(f32r))
        triu_sb = consts.tile([P, P], f16)
        dma(out=triu_sb, in_=t["triu"])
        r128_sb = consts.tile([P, P], f16)
        dma(out=r128_sb, in_=t["r128t"])
        shift_sb = consts.tile([P, 1], f32)
        nc.vector.memset(shift_sb[:], EXP_SHIFT)
        eps_sb = consts.tile([P, 1], f32)
        nc.vector.memset(eps_sb[:], EPS)

        # ================ Stage A: a-projections (kv strip first) ================
        with tc.tile_pool(name="a_sb", bufs=1) as a_sb_pool, \
             tc.tile_pool(name="a_wkv", bufs=16) as a_wkv_pool, \
             tc.tile_pool(name="a_wq", bufs=48) as a_wq_pool, \
             tc.tile_pool(name="a_ev", bufs=3) as a_ev_pool, \
             tc.tile_pool(name="a_ps", bufs=1, space="PSUM") as a_ps_pool, \
             tc.tile_pool(name="a_ssq", bufs=1, space="PSUM") as a_ssq_pool:
            hT_sb = []
            for qtr in range(4):
                ht = a_sb_pool.tile([P, 4, SC], f16, tag=f"hT{qtr}", name=f"hT{qtr}")
                dma(out=ht, in_=t["hT"][512 * qtr:512 * (qtr + 1), :]
                    .rearrange("(t p) s -> p t s", p=P))
                hT_sb.append(ht)

            def hts(k):
                return hT_sb[k // 4][:, k % 4, :]

            ssq_ps = [a_ssq_pool.tile([1, SC], f32, tag=f"ssq{i}", name=f"ssq{i}")
                      for i in range(2)]

            def evict(psum, dst, dst_row, ssq_grp, ssq_start, ssq_stop, rows=P,
                      dt=f16):
                val = a_ev_pool.tile([P, SC], dt, tag=f"val{dt}")
                nc.vector.tensor_copy(val[0:rows, :], psum[0:rows, :])
                dma(out=dst[dst_row:dst_row + rows, :], in_=val[0:rows, :])
                if ssq_grp is not None:
                    sq = a_ev_pool.tile([P, SC], f16, tag="sq")
                    nc.scalar.activation(sq[:], psum[:], AF.Square)
                    MM(ssq_ps[ssq_grp][:], lhsT=ones_col_sb[:], rhs=sq[:],
                       start=ssq_start, stop=ssq_stop)

            # ---- kv strip (5 m-tiles: 4 c_kv + 1 rope, 640 cols) ----
            psums = [a_ps_pool.tile([P, SC], f32, tag=f"aps{i}", name=f"aps{i}")
                     for i in range(5)]
            for k in range(16):
                wt = a_wkv_pool.tile([P, 640], f16, tag="awkv")
                dma(out=wt, in_=t["kvawT_s"][k])
                for ml in range(5):
                    MM(psums[ml][:], lhsT=wt[:, ml * P:(ml + 1) * P],
                       rhs=hts(k), start=(k == 0), stop=(k == 15))
            for ml in range(4):
                evict(psums[ml], t["a_kv_loc"], ml * P, 1, ml == 0, ml == 3)
            evict(psums[4], t["a_kv_loc"], KVL, None, False, False, rows=D_ROPE)
            s_sb = a_ev_pool.tile([1, SC], f16, tag="ssqe")
            nc.vector.tensor_copy(s_sb[:], ssq_ps[1][:])
            dma(out=t["a_kv_loc"][KVL + D_ROPE:KVL + D_ROPE + 1, :], in_=s_sb[:])
            gather(t["a_kv_loc"], t["a_kv"])

            # ---- q strips (3 x 4 m-tiles); gathered once in fp8 ----
            for mg in range(3):
                psums = [a_ps_pool.tile([P, SC], f32, tag=f"aps{i}", name=f"aps{i}")
                         for i in range(4)]
                for k in range(16):
                    wt = a_wq_pool.tile([P, 512], f16, tag="awq")
                    dma(out=wt, in_=t["qawT_s"][k, mg])
                    for ml in range(4):
                        MM(psums[ml][:], lhsT=wt[:, ml * P:(ml + 1) * P],
                           rhs=hts(k), start=(k == 0), stop=(k == 15))
                for ml in range(4):
                    m = mg * 4 + ml
                    evict(psums[ml], t["a_q_loc"], m * P, 0, m == 0, m == 11,
                          dt=f8)
            gather(t["a_q_loc"], t["a_q"])
            s_sb = a_ev_pool.tile([1, SC], f16, tag="ssqe")
            nc.vector.tensor_copy(s_sb[:], ssq_ps[0][:])
            dma(out=t["a_qs_loc"], in_=s_sb[:])
            gather(t["a_qs_loc"], t["a_qs"])

        # ================ persistent tiles + stage B/D weights ================
        qk_stack = ExitStack()
        pers_qk = qk_stack.enter_context(tc.tile_pool(name="pers_qk", bufs=1))
        pers_v = qk_stack.enter_context(tc.tile_pool(name="pers_v", bufs=1))
        q_npT = [pers_qk.tile([P, S], f16, tag=f"qnp{m}", name=f"qnp{m}") for m in range(NHL)]
        q_rpT = [pers_qk.tile([P, S], f16, tag=f"qrp{m}", name=f"qrp{m}") for m in range(2)]
        k_npT = [pers_qk.tile([P, S], f16, tag=f"knp{h}", name=f"knp{h}") for h in range(NHL)]
        k_rpT = pers_qk.tile([P, S], f16, tag="krp", name="krp")
        v_t = [pers_v.tile([P, NHL * D_V], f16, tag=f"v{st}", name=f"v{st}") for st in range(16)]
        pers_at = ec(tc.tile_pool(name="pers_at", bufs=1, side="right"))
        at_onT = [pers_at.tile([P, S], f16, tag=f"aon{h}", name=f"aon{h}") for h in range(NHL)]

        ow_sb = qk_stack.enter_context(tc.tile_pool(name="d_w", bufs=1)).tile([P, 4, H], f16)

        b_stack = ExitStack()
        bpool = b_stack.enter_context(tc.tile_pool(name="bpool", bufs=1))
        cosP_sb = bpool.tile([P, S], f32, tag="cosP")
        dma(out=cosP_sb, in_=t["cosP"])
        sinP_sb = bpool.tile([P, S], f32, tag="sinP")
        dma(out=sinP_sb, in_=t["sinP"])
        kvbw_k_sb = bpool.tile([P, 4, NHL * D_NOPE], f16, tag="kbk")
        dma(out=kvbw_k_sb, in_=t["kvbw_k"].rearrange("(t p) n -> p t n", p=P))
        kvbw_v_sb = bpool.tile([P, 4, NHL * D_V], f16, tag="kbv")
        dma(out=kvbw_v_sb, in_=t["kvbw_v"].rearrange("(t p) n -> p t n", p=P))
        qbw_np_sb = bpool.tile([P, 12, NHL * D_NOPE], f16, tag="qbn")
        dma(out=qbw_np_sb, in_=t["qbw_np"].rearrange("(t p) n -> p t n", p=P))
        qbw_rp_sb = bpool.tile([P, 12, NHL * D_ROPE], f16, tag="qbr")
        dma(out=qbw_rp_sb, in_=t["qbw_rp"].rearrange("(t p) n -> p t n", p=P))
        dma(out=ow_sb, in_=t["owT"].rearrange("(t p) n -> p t n", p=P))

        rdt = t["r_dram"].tensor
        bcast_rk = bpool.tile([P, S], f32, tag="brk")
        rkvT = bpool.tile([P, 16], f32, tag="rkvT")
        bcast_rq = bpool.tile([P, S], f32, tag="brq")

        # ---- rk scale (needs only the kv gather) ----
        # [4,512] shapes (chunk on partitions) keep the DVE reciprocal fast;
        # DMAs go on the Activation HWDGE queue so they don't head-of-line
        # block the kv-pass loads on the Sync queue; tiles live in bpool so
        # no pool-reuse dependencies are created
        sdma = nc.scalar.dma_start
        ssq_kv = bpool.tile([4, SC], f16, tag="skv")
        sdma(out=ssq_kv,
             in_=t["a_kv"][:, KVL + D_ROPE:KVL + D_ROPE + 1, :].rearrange("c r s -> (c r) s"))
        rs_kv = bpool.tile([4, SC], f32, tag="rskv")
        nc.scalar.activation(rs_kv[:], ssq_kv[:], AF.Sqrt,
                             scale=1.0 / KVL, bias=eps_sb[0:4, :])
        rr_kv = bpool.tile([4, SC], f32, tag="rrkv")
        nc.vector.reciprocal_approx_fast(rr_kv[:], rs_kv[:])
        sdma(out=bass.AP(tensor=rdt, offset=S, ap=[[512, 4], [1, SC]]), in_=rr_kv[:])
        sdma(out=bcast_rk, in_=bass.AP(tensor=rdt, offset=S, ap=[[0, P], [1, S]]))
        sdma(out=rkvT, in_=bass.AP(tensor=rdt, offset=S, ap=[[1, P], [P, 16]]))

        # ---- kv-pass: k/v up-projection (overlaps the q gathers) ----
        with tc.tile_pool(name="bk_s", bufs=3) as bk_s, \
             tc.tile_pool(name="bk_t", bufs=2) as bk_t, \
             tc.tile_pool(name="bk_ps", bufs=1, space="PSUM") as bk_ps, \
             tc.tile_pool(name="bk_ps2", bufs=2, space="PSUM") as bk_ps2:
            for c in range(4):
                cols = slice(c * SC, (c + 1) * SC)
                kva_sb = bk_s.tile([P, 4, SC], f16, tag="kva")
                dma(out=kva_sb,
                    in_=t["a_kv"][c, 0:KVL, :].rearrange("(t p) s -> p t s", p=P))
                kps = [bk_ps.tile([P, SC], f32, tag=f"kps{h}", name=f"kps{h}")
                       for h in range(NHL)]
                for f in range(4):
                    for h in range(NHL):
                        MM(kps[h][:], lhsT=kvbw_k_sb[:, f, h * P:(h + 1) * P],
                           rhs=kva_sb[:, f, :], start=(f == 0), stop=(f == 3))
                for h in range(NHL):
                    nc.vector.tensor_mul(k_npT[h][:, cols], kps[h][:], bcast_rk[:, cols])
                for stl in range(4):
                    st = 4 * c + stl
                    vps = bk_ps2.tile([P, NHL * D_V], f32, tag="vps")
                    for f in range(4):
                        MM(vps[:], lhsT=kva_sb[:, f, stl * P:(stl + 1) * P],
                           rhs=kvbw_v_sb[:, f, :], start=(f == 0), stop=(f == 3))
                    nc.vector.tensor_scalar_mul(v_t[st][:], vps[:], rkvT[:, st:st + 1])
                # k_rope (loaded duplicated into both 64-partition halves)
                rp_f = bk_t.tile([P, SC], f16, tag="rp")
                kv_t_ = t["a_kv"].tensor
                rope_off = (c * KV_ROWS + KVL) * SC
                dma(out=rp_f[:],
                    in_=bass.AP(tensor=kv_t_, offset=rope_off,
                                ap=[[0, 2], [SC, D_ROPE], [1, SC]]))
                rot_ps = bk_ps2.tile([P, SC], f32, tag="rotk")
                MM(rot_ps[:], lhsT=r128_sb[:], rhs=rp_f[:], start=True, stop=True)
                t1 = bk_t.tile([P, SC], f32, tag="t1k")
                nc.vector.tensor_mul(t1[:], rot_ps[:], sinP_sb[:, cols])
                t2 = bk_t.tile([P, SC], f32, tag="t2k")
                nc.vector.tensor_mul(t2[:], rp_f[:], cosP_sb[:, cols])
                nc.vector.tensor_add(k_rpT[:, cols], t1[:], t2[:])

        # ---- rq scale (needs the small ssq gather) ----
        ssq_q = bpool.tile([4, SC], f16, tag="sq2")
        sdma(out=ssq_q, in_=t["a_qs"].rearrange("c r s -> (c r) s"))
        rs_q = bpool.tile([4, SC], f32, tag="rsq2")
        nc.scalar.activation(rs_q[:], ssq_q[:], AF.Sqrt,
                             scale=1.0 / QLR, bias=eps_sb[0:4, :])
        rr_q = bpool.tile([4, SC], f32, tag="rrq2")
        nc.vector.reciprocal_approx_fast(rr_q[:], rs_q[:])
        sdma(out=bass.AP(tensor=rdt, offset=0, ap=[[512, 4], [1, SC]]), in_=rr_q[:])
        sdma(out=bcast_rq, in_=bass.AP(tensor=rdt, offset=0, ap=[[0, P], [1, S]]))

        # ---- q-pass: q up-projection + rope for own heads ----
        # one big fp8 load per chunk (cheap Sync descriptor-gen), cast to
        # f16 in f-pairs on the otherwise-idle Vector engine
        with tc.tile_pool(name="bq_s", bufs=2) as bq_s, \
             tc.tile_pool(name="bq_t", bufs=2) as bq_t, \
             tc.tile_pool(name="bq_ps", bufs=1, space="PSUM") as bq_ps, \
             tc.tile_pool(name="bq_ps2", bufs=2, space="PSUM") as bq_ps2:
            for c in range(4):
                cols = slice(c * SC, (c + 1) * SC)
                qa8 = bq_s.tile([P, 12, SC], f8, tag="qa8")
                dma(out=qa8,
                    in_=t["a_q"][c].rearrange("(t p) s -> p t s", p=P))
                qa16 = bq_s.tile([P, 12, SC], f16, tag="qa16")
                qps = [bq_ps.tile([P, SC], f32, tag=f"qps{m}", name=f"qps{m}")
                       for m in range(6)]
                for f in range(12):
                    if f % 2 == 0:
                        nc.vector.tensor_copy(qa16[:, f:f + 2, :], qa8[:, f:f + 2, :])
                    qa_f = qa16[:, f, :]
                    for m in range(4):
                        MM(qps[m][:], lhsT=qbw_np_sb[:, f, m * P:(m + 1) * P],
                           rhs=qa_f, start=(f == 0), stop=(f == 11))
                    for m2 in range(2):
                        MM(qps[4 + m2][:], lhsT=qbw_rp_sb[:, f, m2 * P:(m2 + 1) * P],
                           rhs=qa_f, start=(f == 0), stop=(f == 11))
                for m in range(4):
                    nc.vector.tensor_mul(q_npT[m][:, cols], qps[m][:], bcast_rq[:, cols])
                for m2 in range(2):
                    x_sb = bq_t.tile([P, SC], f16, tag="x")
                    nc.vector.tensor_mul(x_sb[:], qps[4 + m2][:], bcast_rq[:, cols])
                    rot_ps = bq_ps2.tile([P, SC], f32, tag="rot")
                    MM(rot_ps[:], lhsT=r128_sb[:], rhs=x_sb[:], start=True, stop=True)
                    t1 = bq_t.tile([P, SC], f32, tag="t1")
                    nc.vector.tensor_mul(t1[:], rot_ps[:], sinP_sb[:, cols])
                    t2 = bq_t.tile([P, SC], f32, tag="t2")
                    nc.vector.tensor_mul(t2[:], x_sb[:], cosP_sb[:, cols])
                    nc.vector.tensor_add(q_rpT[m2][:, cols], t1[:], t2[:])
        b_stack.close()

        # ================ attention + o-proj, interleaved per Q chunk ================
        c_stack = ExitStack()
        c_pt = c_stack.enter_context(tc.tile_pool(name="c_pt", bufs=6))
        c_da = c_stack.enter_context(tc.tile_pool(name="c_da", bufs=2))
        c_r = c_stack.enter_context(tc.tile_pool(name="c_r", bufs=2))
        d_o = c_stack.enter_context(tc.tile_pool(name="d_o", bufs=2))
        # shared [P,512] f32 PSUM pool: score tiles + o-proj accumulators
        c_sc = c_stack.enter_context(tc.tile_pool(name="c_sc", bufs=4, space="PSUM"))
        c_at = c_stack.enter_context(tc.tile_pool(name="c_at", bufs=3, space="PSUM"))
        c_dn = c_stack.enter_context(tc.tile_pool(name="c_dn", bufs=1, space="PSUM"))
        rec_t = t["rec_dram"].tensor

        def attn_head_chunk(h, Q):
            rp_tile = q_rpT[h // 2]
            rp_lo = D_ROPE * (h % 2)
            qcols = slice(Q * 512, (Q + 1) * 512)
            at_ps = c_at.tile([P, 512], f32, tag="at")
            dacc = c_da.tile([P, 512], f16, tag="dacc")
            jmax = 4 * Q + 3
            pend = []  # software pipeline: AV matmul for j lags scores by two
            for j in range(jmax + 1):
                jp = j - 4 * Q
                lo = max(jp, 0) * P
                qsl = slice(Q * 512 + lo, (Q + 1) * 512)
                ksl = slice(j * P, (j + 1) * P)
                sc_ps = c_sc.tile([P, 512], f32, tag="sc")
                MM(sc_ps[:, lo:], lhsT=k_npT[h][:, ksl], rhs=q_npT[h][:, qsl],
                   start=True, stop=False)
                MM(sc_ps[:, lo:], lhsT=k_rpT[rp_lo:rp_lo + D_ROPE, ksl],
                   rhs=rp_tile[rp_lo:rp_lo + D_ROPE, qsl],
                   start=False, stop=True)
                pt = c_pt.tile([P, 512], f16, tag="pt")
                nc.scalar.activation(pt[:, lo:], sc_ps[:, lo:], AF.Exp,
                                     scale=SCALING, bias=shift_sb[:])
                if jp >= 0:
                    nc.vector.tensor_mul(pt[:, lo:lo + P], pt[:, lo:lo + P],
                                         triu_sb[:])
                if j == 0:
                    nc.vector.tensor_copy(dacc[:], pt[:])
                else:
                    nc.vector.tensor_add(dacc[:, lo:], dacc[:, lo:], pt[:, lo:])
                pend.append((j, lo, pt))
                if len(pend) > 2:
                    pj, plo, ppt = pend.pop(0)
                    MM(at_ps[:, plo:], lhsT=v_t[pj][:, h * D_V:(h + 1) * D_V],
                       rhs=ppt[:, plo:], start=(pj == 0), stop=False)
            for pj, plo, ppt in pend:
                MM(at_ps[:, plo:], lhsT=v_t[pj][:, h * D_V:(h + 1) * D_V],
                   rhs=ppt[:, plo:], start=(pj == 0), stop=(pj == jmax))
            # denominator: partition-reduce the vector-accumulated dacc, then
            # broadcast 1/dn across partitions via a DRAM round-trip (stride-0
            # partition read) so the Tensor queue never stalls on this tail
            dn_ps = c_dn.tile([1, 512], f32, tag="dn")
            MM(dn_ps[:], lhsT=ones_col_sb[:], rhs=dacc[:], start=True, stop=True)
            rec = c_r.tile([1, 512], f32, tag="rec")
            nc.vector.reciprocal_approx_fast(rec[:], dn_ps[:])
            hq = h * 4 + Q
            dma(out=t["rec_dram"][hq:hq + 1, :], in_=rec[:])
            bc_sb = c_r.tile([P, 512], f32, tag="bcs")
            dma(out=bc_sb, in_=bass.AP(tensor=rec_t, offset=hq * 512,
                                       ap=[[0, P], [1, 512]]))
            nc.vector.tensor_mul(at_onT[h][:, qcols], at_ps[:], bc_sb[:])

        def oproj_chunk(Q):
            for qtl in range(4):
                qt = 4 * Q + qtl
                out_sb = d_o.tile([P, H], f16, tag="out")
                for hc in range(4):
                    psum = c_sc.tile([P, 512], f32, tag="sc")
                    for f in range(4):
                        MM(psum[:], lhsT=at_onT[f][:, qt * P:(qt + 1) * P],
                           rhs=ow_sb[:, f, hc * 512:(hc + 1) * 512],
                           start=(f == 0), stop=(f == 3))
                    nc.vector.tensor_copy(out_sb[:, hc * 512:(hc + 1) * 512], psum[:])
                dma(out=t["o_part"][qt * P:(qt + 1) * P, :], in_=out_sb[:])

        for Q in range(4):
            for h in range(NHL):
                attn_head_chunk(h, Q)
            oproj_chunk(Q)

        c_stack.close()
        qk_stack.close()


# ---------------- host side ----------------
_CACHED = {}


def _get_program():
    if "nc" not in _CACHED:
        _CACHED["nc"] = _build_program()
    return _CACHED["nc"]


def _host_consts():
    inv_freq = 1.0 / (ROPE_THETA ** (np.arange(0, D_ROPE, 2, dtype=np.float32) / D_ROPE))
    ti = np.arange(S, dtype=np.float32)
    ang = np.outer(ti, inv_freq)
    emb = np.concatenate([ang, ang], axis=-1)          # [S, 64]
    cosT = np.cos(emb).T.astype(np.float32)            # [64, S]
    sinT = np.sin(emb).T.astype(np.float32)
    cosP = np.vstack([cosT, cosT])                     # [128, S]
    sinP = np.vstack([sinT, sinT])
    r64 = np.zeros((D_ROPE, D_ROPE), np.float16)
    hlf = D_ROPE // 2
    for i in range(hlf):
        r64[i, i + hlf] = -1.0
        r64[i + hlf, i] = 1.0
    r128 = np.zeros((P, P), np.float16)
    r128[:D_ROPE, :D_ROPE] = r64
    r128[D_ROPE:, D_ROPE:] = r64
    r128t = np.ascontiguousarray(r128.T)
    kk, qq = np.meshgrid(np.arange(P), np.arange(P), indexing="ij")
    triu = (kk <= qq).astype(np.float16)
    return cosP, sinP, r128t, triu


def make_in_maps(hidden_states, q_a_w, q_a_ln_w, q_b_w, kv_a_w, kv_a_ln_w,
                 kv_b_w, o_w):
    f, f16_ = np.float32, np.float16
    hidden_states = np.asarray(hidden_states, f)
    q_b_eff = (np.asarray(q_b_w, f) * np.asarray(q_a_ln_w, f)[None, :]).astype(f16_)
    kv_b_eff = (np.asarray(kv_b_w, f) * np.asarray(kv_a_ln_w, f)[None, :]).astype(f16_)
    qawT = np.asarray(q_a_w, f).T.astype(f16_)         # [H, QLR]
    kvawT_pad = np.zeros((H, 5 * P), f16_)
    kvawT_pad[:, :KVL + D_ROPE] = np.asarray(kv_a_w, f).T.astype(f16_)
    qawT_s = np.ascontiguousarray(
        qawT.reshape(16, P, 3, 512).transpose(0, 2, 1, 3))
    kvawT_s = np.ascontiguousarray(kvawT_pad.reshape(16, P, 640))
    cosP, sinP, r128t, triu = _host_consts()
    ones_col = np.ones((P, 1), f16_)
    ones_row = np.ones((1, P), f)

    in_maps = []
    for core in range(N_CORES):
        b, g = divmod(core, 4)
        heads = range(NHL * g, NHL * (g + 1))
        hT = np.ascontiguousarray(hidden_states[b, g * SC:(g + 1) * SC, :].T.astype(f16_))
        qbw_np = np.ascontiguousarray(np.concatenate(
            [q_b_eff[D_QK * hh:D_QK * hh + D_NOPE] for hh in heads], 0).T)
        qbw_rp = np.ascontiguousarray(np.concatenate(
            [q_b_eff[D_QK * hh + D_NOPE:D_QK * (hh + 1)] for hh in heads], 0).T)
        kvbw_k = np.ascontiguousarray(np.concatenate(
            [kv_b_eff[(D_NOPE + D_V) * hh:(D_NOPE + D_V) * hh + D_NOPE]
             for hh in heads], 0).T)
        kvbw_v = np.ascontiguousarray(np.concatenate(
            [kv_b_eff[(D_NOPE + D_V) * hh + D_NOPE:(D_NOPE + D_V) * (hh + 1)]
             for hh in heads], 0).T)
        owT = np.ascontiguousarray(
            np.asarray(o_w, f)[:, g * NHL * D_V:(g + 1) * NHL * D_V].T.astype(f16_))
        in_maps.append(dict(
            hT=hT, qawT_s=qawT_s, kvawT_s=kvawT_s, qbw_np=qbw_np,
            qbw_rp=qbw_rp, kvbw_k=kvbw_k, kvbw_v=kvbw_v, owT=owT,
            cosP=cosP, sinP=sinP, r128t=r128t, triu=triu,
            ones_col=ones_col, ones_row=ones_row))
    return in_maps


def kernel(**inputs):
    nc = _get_program()
    in_maps = make_in_maps(**inputs)
    res = run_bass_kernel_spmd(nc, in_maps, core_ids=list(range(N_CORES)))
    out = np.zeros((B, S, H), np.float32)
    for core in range(N_CORES):
        out[core // 4] += res.results[core]["o_part"].astype(np.float32)
    return out
